# revision 1
# baseline (speedup 1.0000x reference)
"""Trainium2 Bass kernel for nn_CustomSelfAttention_24257975288159.

Reference computation (B=4, L=2048, D=1024, H=16, HD=64, fp32):
  q = x @ Wq + bq ; q[:, 1, :] = cross_cls_sent @ Wq + bq
  k = x @ Wk + bk ; v = x @ Wv + bv
  out = softmax(q k^T / sqrt(HD) + mask) v       (per head)

Sharding: 8 cores = batch (4) x head-group (2). Core c handles batch
c//2 and heads 8*(c%2)..8*(c%2)+7 (columns 512*(c%2)..+512 of the model
dim); QKV weights are column-sharded per head group.

v2 design. HW-measured facts this is built on: the ScalarE exp of a
[128,1024] tile costs 1165ns and the 256 of them (~298us) are the
per-core floor; row-tiled K=64 matmul pairs at tile_position (0,0) /
(64,0) run CONCURRENTLY (296ns/pair); back-to-back N=512 matmuls
pipeline at ~228ns each; M=65 ctx pairs serialize at ~470ns/pair.
  - All matmul operands in bf16 (x, W, q, k, v, probs); psum stays f32.
  - x is transposed into xT [d-part, l-free] by XBAR DMA-transpose
    straight from DRAM (no PE transposes, no staging); SP queue only
    (ACT-queue DMA-transposes have broken completion sync -> races).
  - Scores per step (pair p, lq-quarter q, lk-chunk c): two K=64
    row-tiled matmuls (concurrent), one fused exp(SCALE*s + mask) on
    ACT covering both heads -> pt bf16 (16 rotating buffers so exp can
    run far ahead of the v-gated early ctx chain), two M=65 ctx
    matmuls (v carries a ones column so the same matmul accumulates
    softmax denominators).
  - q/k/v projection and denominator-reciprocal work is split into
    small units emitted between attention steps from a deadline-sorted
    queue (forced out just before their first consumer, budget-paced
    otherwise) so the in-order PE queue never bursts long enough to
    starve ACT.
  - Finish per quartet: ctxT (bf16) strips transposed back by
    SBUF->SBUF DMA-transpose, scaled by reciprocal denominators on the
    otherwise-idle GPSIMD engine (keeps DVE from blocking behind DMA
    latency), DMA'd out.
Measured (hw_loop differencing, 8 cores): ~430-450us/iter vs ~540-625
for the v1 baseline (same method; the staged 1265us figure was the
previous session's single-shot estimate).
"""
import numpy as np

import concourse.bass as bass
import concourse.mybir as mybir
import concourse.tile as tile

F32 = mybir.dt.float32
BF16 = mybir.dt.bfloat16

B, L, D, H = 4, 2048, 1024, 16
HD = D // H          # 64
SCALE = float(1.0 / np.sqrt(HD))
DG = D // 2          # 512 output columns per core (8 heads)
NCORES = 8
LC = L // 128        # 16 lk-chunks
DC = D // 128        # 8 d-chunks
GC = DG // 128       # 4 dh-chunks per core = head pairs
HS = HD + 1          # 65: v columns per head incl ones column

_CACHED = {}


# ---------------------------------------------------------------------------
# walrus workarounds:
#  - this build rejects >1 sync-wait per instruction -> spill onto NOPs
#  - InstDmaTransposeAnt cannot carry sync waits at all -> spill all of them
# ---------------------------------------------------------------------------
def _split_excess_waits(nc, max_waits=1):
    counter = 0
    for fn in nc.m.functions:
        for blk in fn.blocks:
            il = blk.instructions
            out = []
            changed = False
            for ins in il:
                si = getattr(ins, "sync_info", None)
                waits = list(si.on_wait) if si is not None and si.on_wait else []
                limit = 0 if isinstance(ins, mybir.InstDmaTransposeAnt) else max_waits
                if len(waits) > limit:
                    si.on_wait = waits[:limit]
                    spill = waits[limit:]
                    for i in range(0, len(spill), max_waits):
                        counter += 1
                        out.append(
                            mybir.InstNoOp(
                                name=f"waitsplit_{counter}",
                                engine=ins.engine,
                                bass_nofuse=True,
                                sync_info=mybir.SyncInfo(
                                    on_wait=spill[i:i + max_waits], on_update=[]
                                ),
                            )
                        )
                    changed = True
                out.append(ins)
            if changed:
                il.clear()
                il.extend(out)
    return counter


def _build_program(repeat=1, hw_loop=0):
    nc = bass.Bass()

    x_d = nc.declare_dram_parameter("x", [L, D], BF16, isOutput=False)
    wq_d = nc.declare_dram_parameter("wq", [D, DG], BF16, isOutput=False)
    wk_d = nc.declare_dram_parameter("wk", [D, DG], BF16, isOutput=False)
    wv_d = nc.declare_dram_parameter("wv", [D, DG], BF16, isOutput=False)
    qc_d = nc.declare_dram_parameter("qcross", [128, GC], BF16, isOutput=False)
    bq_d = nc.declare_dram_parameter("bq", [128, GC], F32, isOutput=False)
    bk_d = nc.declare_dram_parameter("bk", [128, GC], F32, isOutput=False)
    bv_d = nc.declare_dram_parameter("bv", [1, DG], F32, isOutput=False)
    mk_d = nc.declare_dram_parameter("maskm", [128, LC], F32, isOutput=False)
    out_d = nc.declare_dram_parameter("out", [L, DG], F32, isOutput=True)

    with tile.TileContext(nc, pool_alloc_mode="queue") as tc:
        with (
            tc.tile_pool(name="const", bufs=1) as const,
            tc.tile_pool(name="qkv", bufs=1) as qkv,
            tc.tile_pool(name="xt", bufs=1) as xtp,
            tc.tile_pool(name="wts", bufs=1) as wtsp,
            tc.tile_pool(name="pt", bufs=1) as ptp,
            tc.tile_pool(name="fin", bufs=1) as finp,
            tc.tile_pool(name="fps", bufs=1) as fpsp,
            tc.tile_pool(name="ost", bufs=4) as ostp,
        ):
            # ---- constants ----
            ones1 = const.tile([1, DG], F32)
            nc.vector.memset(ones1[:], 1.0)
            ones8 = const.tile([128, 8], BF16)
            nc.vector.memset(ones8[:], 1.0)
            qc_sb = const.tile([128, GC], BF16)
            nc.sync.dma_start(out=qc_sb[:], in_=qc_d[:, :])
            bq_sb = const.tile([128, GC], F32)
            nc.sync.dma_start(out=bq_sb[:], in_=bq_d[:, :])
            bk_sb = const.tile([128, GC], F32)
            nc.sync.dma_start(out=bk_sb[:], in_=bk_d[:, :])
            bv_sb = const.tile([1, DG], F32)
            nc.sync.dma_start(out=bv_sb[:], in_=bv_d[:, :])
            mk_sb = const.tile([128, LC], F32)
            nc.sync.dma_start(out=mk_sb[:], in_=mk_d[:, :])
            bias_v = const.tile([128, DG], F32)

            def body():
                xT = [
                    xtp.tile([128, L], BF16, tag=f"xT{dc}", name=f"xT{dc}")
                    for dc in range(DC)
                ]
                vt = [
                    qkv.tile([128, 8 * HS], BF16, tag=f"v{lc}", name=f"v{lc}")
                    for lc in range(LC)
                ]
                ctxT = [
                    finp.tile([128, L], BF16, tag=f"ctxT{p}", name=f"ctxT{p}")
                    for p in range(GC)
                ]
                # denominator rows at 32-aligned partitions; pair p uses rows
                # 64*(p%2) and 64*(p%2)+32
                den = finp.tile([97, L], F32, tag="den")
                nc.vector.memset(den[:], 1.0)
                sel = finp.tile([97, 4], F32, tag="sel")
                nc.vector.memset(sel[:], 0.0)
                nc.vector.memset(sel[0:1, 0:1], 1.0)
                nc.vector.memset(sel[32:33, 1:2], 1.0)
                nc.vector.memset(sel[64:65, 2:3], 1.0)
                nc.vector.memset(sel[96:97, 3:4], 1.0)
                rcpT = [
                    finp.tile([128, 2 * LC], F32, tag=f"rcp{p}", name=f"rcp{p}")
                    for p in range(GC)
                ]

                # ---- weights: one 3D-AP DMA per (pair, q/k) column block ----
                def wtiles(wd, p, pfx):
                    # [128 d-part, (dc, col)]: all 8 d-chunks of W[:, p*128:+128]
                    t = wtsp.tile(
                        [128, DC * 128], BF16, tag=f"{pfx}{p}", name=f"{pfx}{p}"
                    )
                    nc.sync.dma_start(
                        out=t.rearrange("p (dc c) -> p dc c", c=128),
                        in_=wd[:, p * 128:(p + 1) * 128].rearrange(
                            "(dc p) c -> p dc c", p=128
                        ),
                    )
                    return [t[:, dc * 128:(dc + 1) * 128] for dc in range(DC)]

                # xT transposes first: the k0 projection gates everything.
                # One whole-column-block transpose per dc (fewer HWDGE slots).
                wk_t = {0: wtiles(wk_d, 0, "wk")}
                for dc in range(DC):
                    # alternate the two HWDGE queues (SP/ACT) so the x
                    # transposes stream in parallel; ACT is idle here
                    eng = nc.sync
                    eng.dma_start_transpose(
                        out=xT[dc][:],
                        in_=x_d[:, dc * 128:(dc + 1) * 128],
                    )
                wq_t = {0: wtiles(wq_d, 0, "wq")}
                wv_t = []
                for dc in range(DC):
                    t = wtsp.tile([128, DG], BF16, tag=f"wv{dc}", name=f"wv{dc}")
                    nc.sync.dma_start(
                        out=t[:], in_=wv_d[dc * 128:(dc + 1) * 128, :]
                    )
                    wv_t.append(t)
                for p in range(1, GC):
                    wk_t[p] = wtiles(wk_d, p, "wk")
                    wq_t[p] = wtiles(wq_d, p, "wq")

                with tc.tile_pool(name="psBC", bufs=1, space="PSUM") as psBC:
                    with nc.named_scope("biasv"):
                        psb = psBC.tile([128, DG], F32, tag="proj", bufs=2)
                        nc.tensor.matmul(
                            psb[:], ones1[0:1, 0:128], bv_sb[:],
                            start=True, stop=True,
                        )
                        nc.vector.tensor_copy(bias_v[:], psb[:])

                    qk = {}

                    def get_qk(p):
                        if p not in qk:
                            qk[p] = (
                                qkv.tile([128, L], BF16, tag="qTs", bufs=3,
                                         name=f"qTs{p}"),
                                qkv.tile([128, L], BF16, tag="kTs", bufs=3,
                                         name=f"kTs{p}"),
                            )
                        return qk[p]

                    def u_projw(p, w, isq):
                        # one lq-window of the q or k projection for pair p
                        dst = get_qk(p)[0 if isq else 1]
                        wt = (wq_t if isq else wk_t)[p]
                        bias_sb = bq_sb if isq else bk_sb
                        with nc.named_scope(f"proj{'q' if isq else 'k'}{p}_{w}"):
                            psw = psBC.tile([128, 512], F32, tag="proj", bufs=2)
                            for dc in range(DC):
                                nc.tensor.matmul(
                                    psw[:],
                                    wt[dc],
                                    xT[dc][:, w * 512:(w + 1) * 512],
                                    start=(dc == 0),
                                    stop=(dc == DC - 1),
                                )
                            nc.vector.tensor_scalar_add(
                                dst[:, w * 512:(w + 1) * 512],
                                psw[:],
                                bias_sb[:, p:p + 1],
                            )
                            if isq and w == 0:
                                # CLS_sent query substitution at row 1
                                nc.vector.tensor_copy(
                                    dst[:, 1:2], qc_sb[:, p:p + 1]
                                )

                    def u_v(lc):
                        with nc.named_scope(f"v{lc}"):
                            psv = psBC.tile([128, 512], F32, tag="proj", bufs=2)
                            for dc in range(DC):
                                nc.tensor.matmul(
                                    psv[:],
                                    xT[dc][:, lc * 128:(lc + 1) * 128],
                                    wv_t[dc][:],
                                    start=(dc == 0),
                                    stop=(dc == DC - 1),
                                )
                            v = vt[lc]
                            ones_cols = v.rearrange("p (h s) -> p h s", s=HS)[
                                :, :, HD
                            ]
                            nc.vector.tensor_copy(ones_cols, ones8[:])
                            nc.vector.tensor_add(
                                v.rearrange("p (h s) -> p h s", s=HS)[:, :, 0:HD],
                                psv.rearrange("p (h s) -> p h s", s=HD),
                                bias_v.rearrange("p (h s) -> p h s", s=HD),
                            )

                    def u_rt(p, qq):
                        # reciprocal denominators for quartet qq of pair p
                        with nc.named_scope(f"rt{p}_{qq}"):
                            rt = psBC.tile(
                                [128, 8], F32, tag="proj", bufs=2,
                                name=f"rt{p}_{qq}",
                            )
                            sc = 2 * (p % 2)
                            for j in range(4):
                                lc = 4 * qq + j
                                nc.tensor.transpose(
                                    rt[:, j * 2:j * 2 + 2],
                                    den[0:97, lc * 128:(lc + 1) * 128],
                                    sel[0:97, sc:sc + 2],
                                )
                            nc.vector.reciprocal(
                                rcpT[p][:, 8 * qq:8 * qq + 8], rt[:]
                            )

                    def u_strip(p, lc):
                        fpsT = fpsp.tile(
                            [128, 128], BF16, tag="fps", bufs=4,
                            name=f"fps_{p}_{lc}",
                        )
                        nc.sync.dma_start_transpose(
                            out=fpsT[:],
                            in_=ctxT[p][:, lc * 128:(lc + 1) * 128],
                        )
                        ost = ostp.tile(
                            [128, 128], F32, tag="ost", name=f"ost_{p}_{lc}"
                        )
                        # on GPSIMD: keeps the in-order DVE queue from
                        # blocking behind the transpose-DMA latency
                        for h in range(2):
                            nc.gpsimd.tensor_scalar_mul(
                                ost[:, h * 64:h * 64 + 64],
                                fpsT[:, h * 64:h * 64 + 64],
                                rcpT[p][:, lc * 2 + h:lc * 2 + h + 1],
                            )
                        nc.sync.dma_start(
                            out=out_d[
                                lc * 128:(lc + 1) * 128,
                                p * 128:(p + 1) * 128,
                            ],
                            in_=ost[:],
                        )

                    def scores(s_):
                        p_, r = divmod(s_, 4 * LC)
                        q_, c_ = divmod(r, LC)
                        q_t, k_t = get_qk(p_)
                        lq = q_ * 512
                        sAB = psBC.tile(
                            [128, 1024], F32, tag="sAB", bufs=2,
                            name=f"sAB_{p_}_{q_}_{c_}",
                        )
                        nc.tensor.matmul(
                            sAB[:, 0:512],
                            k_t[0:64, c_ * 128:(c_ + 1) * 128],
                            q_t[0:64, lq:lq + 512],
                            start=True, stop=True,
                            tile_position=(0, 0),
                        )
                        nc.tensor.matmul(
                            sAB[:, 512:1024],
                            k_t[64:128, c_ * 128:(c_ + 1) * 128],
                            q_t[64:128, lq:lq + 512],
                            start=True, stop=True,
                            tile_position=(64, 0),
                        )
                        return sAB

                    # ---- preamble: k0 fully, q0 window 0 ----
                    for w in range(4):
                        u_projw(0, w, isq=False)
                    u_projw(0, 0, isq=True)

                    # ---- deadline-sorted filler queue ----
                    # each entry: (deadline_step, kind, arg). A unit MUST be
                    # emitted at or before its deadline (correctness for v:
                    # ctx(0,0,c) reads vt[c] at step c; proj pair p windows
                    # before the scores that read them).
                    UNIT_NS = {"projw": 2000, "q0w": 2000, "v": 2000,
                               "rt": 500}
                    # deadline = step of the earliest consumer's emission;
                    # pops run before the scores/ctx emissions of each step
                    fillers = [(lc, "v", lc) for lc in range(LC)]
                    fillers += [(16 * w - 2, "q0w", w) for w in (1, 2, 3)]
                    for p in range(1, GC):
                        for w in range(4):
                            fillers.append(
                                (64 * p + 4 * w - 2, "projw", (p, w, False)))
                        for w in range(4):
                            fillers.append(
                                (64 * p + 16 * w - 2, "projw", (p, w, True)))
                    fillers.sort(key=lambda u: u[0])
                    strips = []

                    def emit_unit(u):
                        _, kind, arg = u
                        if kind == "v":
                            u_v(arg)
                        elif kind == "q0w":
                            u_projw(0, arg, isq=True)
                        elif kind == "projw":
                            p_, w_, isq_ = arg
                            u_projw(p_, w_, isq_)
                        elif kind == "rt":
                            p_, qq_ = arg
                            u_rt(p_, qq_)
                            # strips read rcpT[p]; only runnable after rt
                            for j in range(4):
                                strips.append((p_, 4 * qq_ + j))

                    # ---- flat attention pipeline ----
                    NSTEP = GC * 4 * LC
                    LOOKAHEAD = 2

                    sABs = {}
                    for s_ in range(LOOKAHEAD):
                        sABs[s_] = scores(s_)
                    cA = cB = None
                    budget = 0.0
                    for s_ in range(NSTEP):
                        p, r = divmod(s_, 4 * LC)
                        q, c = divmod(r, LC)
                        if c == 0:
                            cA = psBC.tile([65, 512], F32, tag="ctxA",
                                           name=f"cA{p}_{q}")
                            cB = psBC.tile([65, 512], F32, tag="ctxB",
                                           name=f"cB{p}_{q}")
                        # filler units run in the PE gap while ctx waits on
                        # exp. Units at their deadline are forced out BEFORE
                        # the scores/ctx emissions that consume their output;
                        # otherwise budget-paced to avoid starving ACT.
                        budget = min(budget + 400.0, 2000.0)
                        while fillers and fillers[0][0] <= s_:
                            budget = 0.0
                            emit_unit(fillers.pop(0))
                        if fillers and UNIT_NS[fillers[0][1]] <= budget:
                            u = fillers.pop(0)
                            budget -= UNIT_NS[u[1]]
                            emit_unit(u)
                        if s_ + LOOKAHEAD < NSTEP:
                            sABs[s_ + LOOKAHEAD] = scores(s_ + LOOKAHEAD)
                        sAB = sABs.pop(s_)
                        pt = ptp.tile([128, 1024], BF16, tag="pt", bufs=16)
                        nc.scalar.activation(
                            pt[:],
                            sAB[:],
                            mybir.ActivationFunctionType.Exp,
                            bias=mk_sb[:, c:c + 1],
                            scale=SCALE,
                        )
                        if strips:
                            u_strip(*strips.pop(0))
                        hA = 2 * p * HS
                        hB = (2 * p + 1) * HS
                        nc.tensor.matmul(
                            cA[:],
                            vt[c][:, hA:hA + HS],
                            pt[:, 0:512],
                            start=(c == 0), stop=(c == LC - 1),
                        )
                        nc.tensor.matmul(
                            cB[:],
                            vt[c][:, hB:hB + HS],
                            pt[:, 512:1024],
                            start=(c == 0), stop=(c == LC - 1),
                        )
                        if c == LC - 1:
                            lq = q * 512
                            nc.vector.tensor_copy(
                                ctxT[p][0:64, lq:lq + 512], cA[0:64, :]
                            )
                            nc.vector.tensor_copy(
                                ctxT[p][64:128, lq:lq + 512], cB[0:64, :]
                            )
                            pb = 64 * (p % 2)
                            nc.vector.tensor_copy(
                                den[pb:pb + 1, lq:lq + 512], cA[64:65, :]
                            )
                            nc.vector.tensor_copy(
                                den[pb + 32:pb + 33, lq:lq + 512], cB[64:65, :]
                            )
                            fillers.insert(0, (s_, "rt", (p, q)))

                    # ---- drain remaining work ----
                    while fillers:
                        emit_unit(fillers.pop(0))
                    while strips:
                        u_strip(*strips.pop(0))

            if hw_loop:
                with tc.For_i(0, hw_loop, 1):
                    body()
            else:
                for _rep in range(repeat):
                    body()

    _split_excess_waits(nc)
    return nc


def _bf16(a):
    import ml_dtypes

    return np.ascontiguousarray(np.asarray(a, dtype=np.float32)).astype(
        ml_dtypes.bfloat16
    )


def make_in_maps(inputs):
    x = np.asarray(inputs["x"], dtype=np.float32)
    attn_mask = np.asarray(inputs["attn_mask"], dtype=np.float32)
    cross = np.asarray(inputs["cross_cls_sent"], dtype=np.float32)
    Wq = np.asarray(inputs["Wq"], dtype=np.float32)
    bq = np.asarray(inputs["bq"], dtype=np.float32)
    Wk = np.asarray(inputs["Wk"], dtype=np.float32)
    bk = np.asarray(inputs["bk"], dtype=np.float32)
    Wv = np.asarray(inputs["Wv"], dtype=np.float32)
    bv = np.asarray(inputs["bv"], dtype=np.float32)

    in_maps = []
    for c in range(NCORES):
        b = c // 2
        g = c % 2
        cols = slice(g * DG, (g + 1) * DG)
        qcross = cross[b] @ Wq[:, cols] + bq[cols]  # host matvec
        in_maps.append(
            {
                "x": _bf16(x[b]),
                "wq": _bf16(Wq[:, cols]),
                "wk": _bf16(Wk[:, cols]),
                "wv": _bf16(Wv[:, cols]),
                "qcross": _bf16(qcross.reshape(GC, 128).T),
                "bq": np.ascontiguousarray(bq[cols].reshape(GC, 128).T),
                "bk": np.ascontiguousarray(bk[cols].reshape(GC, 128).T),
                "bv": np.ascontiguousarray(bv[cols].reshape(1, DG)),
                "maskm": np.ascontiguousarray(
                    attn_mask[b, 0, 0].reshape(LC, 128).T
                ),
            }
        )
    return in_maps


def kernel(x, attn_mask, cross_cls_sent, Wq, bq, Wk, bk, Wv, bv):
    from concourse.bass_utils import run_bass_kernel_spmd

    if "nc" not in _CACHED:
        _CACHED["nc"] = _build_program()
    nc = _CACHED["nc"]

    in_maps = make_in_maps(
        {
            "x": x,
            "attn_mask": attn_mask,
            "cross_cls_sent": cross_cls_sent,
            "Wq": Wq,
            "bq": bq,
            "Wk": Wk,
            "bk": bk,
            "Wv": Wv,
            "bv": bv,
        }
    )

    res = run_bass_kernel_spmd(nc, in_maps, list(range(NCORES)))
    out = np.empty((B, L, D), dtype=np.float32)
    for c in range(NCORES):
        b = c // 2
        g = c % 2
        out[b][:, g * DG:(g + 1) * DG] = res.results[c]["out"]
    return out

